# revision 1
# baseline (speedup 1.0000x reference)
"""LoRA attention processor on 8 NeuronCores (Trainium2, Bass/Tile).

Reference computation (B=2, S=4096, D=1280, H=8 heads, dh=160, rank-4 LoRA
on K/V):
    q = x @ Wq; k = x @ Wk; v = x @ Wv
    k += (k @ Ak) @ Bk; v += (v @ Av) @ Bv        (LoRA, rank 4)
    attn = softmax(q k^T / sqrt(dh)) v   per head
    out = attn @ Wout + b_out

Sharding: core c handles batch b = c//4 and head pair p = c%4 (columns
320p:320p+320 of the QKV projections, rows of Wout). The LoRA update is
folded into the weights on the host: k + (k@Ak)@Bk == x @ (Wk + Wk@Ak@Bk),
so each core only needs its 320-column slice of the effective weights.
Each core returns a partial output (its heads' contribution to attn@Wout);
the host sums the 4 partials per batch and adds the bias.

On-core layout: scores are computed transposed ([k-pos partitions, q-pos
free]) so softmax's exp runs on ACT over PSUM directly and the PV matmul
needs no transposes: outT[d, q] = sum_j V[j, d] * expT[j, q]. The softmax
denominator rides along as a ones-column appended to V (row 160 of the PV
output), and normalization is applied to outT (160 x 4096 per head)
instead of to the 4096 x 4096 probability matrix. No row-max subtraction:
scores are ~N(0,1) here (|s| < ~7), exp cannot overflow fp32.

All big matmuls run in float32r (TF32-style reduced-precision fp32, full
PE rate at free-dim >= 256 vs 4x slower for exact fp32).
"""

import numpy as np
import ml_dtypes
from contextlib import ExitStack

import concourse.bass as bass
import concourse.tile as tile
from concourse import bacc, mybir
from concourse.bass_utils import run_bass_kernel_spmd

B, S, D = 2, 4096, 1280
H, DH = 8, 160
HP = 320           # head-pair columns per core (2 heads)
N_CORES = 8
SC = 512           # free-dim chunk (q columns / s columns)
NSC = S // SC      # 8
CK = 128           # contraction chunk
NCK = D // CK      # 10
F32 = mybir.dt.float32
F32R = mybir.dt.float32r
BF16 = mybir.dt.bfloat16

_CACHE = {}


def build():
    nc = bacc.Bacc("TRN2", target_bir_lowering=False, debug=False,
                   num_devices=N_CORES)
    # inputs (float32r decl == fp32 bits; PE reads reduced precision)
    xT = nc.dram_tensor("xT", [D, S], F32R, kind="ExternalInput").ap()
    wq = nc.dram_tensor("wq", [D, HP], F32R, kind="ExternalInput").ap()
    wk = nc.dram_tensor("wk", [D, HP], F32R, kind="ExternalInput").ap()
    wv = nc.dram_tensor("wv", [D, HP], F32R, kind="ExternalInput").ap()
    wo = nc.dram_tensor("wo", [HP, D], F32R, kind="ExternalInput").ap()
    ones2 = nc.dram_tensor("ones2", [1, 128], F32, kind="ExternalInput").ap()
    # [...,0]=1 feeds the denominator row of the PV matmul; [...,1]=0 pads
    # the V free dim to an even size (fp32r layout rule)
    onesv = nc.dram_tensor("onesv", [128, 32, 2], F32R, kind="ExternalInput").ap()
    out = nc.dram_tensor("out", [S, D], F32, kind="ExternalOutput").ap()
    # scratch
    qT_d = nc.dram_tensor("qT_d", [HP, S], F32R).ap()
    oT_d = nc.dram_tensor("oT_d", [HP, S], F32R).ap()

    # per-head row chunks of the 320-wide slice: (offset, size)
    hchunks = [[(0, 128), (128, 32)], [(160, 128), (288, 32)]]

    with tile.TileContext(nc) as tc, ExitStack() as top:
        kt_pool = top.enter_context(tc.tile_pool(name="kt", bufs=1))
        v_pool = top.enter_context(tc.tile_pool(name="vp", bufs=1))
        KT = [kt_pool.tile([sz, S], F32R, name=f"KT{i}", tag=f"KT{i}")
              for i, (_, sz) in enumerate(hchunks[0] + hchunks[1])]
        V = [v_pool.tile([128, 32, 162], F32R, name=f"V{h}", tag=f"V{h}")
             for h in range(2)]

        # ---- phase 1: projections QT/KT (transposed) + V (natural) ----
        with ExitStack() as ph1:
            xp = ph1.enter_context(tc.tile_pool(name="xp", bufs=2))
            wp = ph1.enter_context(tc.tile_pool(name="wp", bufs=1))
            pp = ph1.enter_context(tc.tile_pool(name="pp", bufs=4, space="PSUM"))
            sp = ph1.enter_context(tc.tile_pool(name="sp", bufs=3))

            warm = sp.tile([1, 2], F32, tag="warm")
            nc.vector.memset(warm[:], 0.0)
            warm2 = sp.tile([1, 2], F32, tag="warm2")
            nc.scalar.activation(warm2[:], warm[:],
                                 mybir.ActivationFunctionType.Exp)
            wts = {}
            for nm, src in (("wq", wq), ("wk", wk), ("wv", wv)):
                for c in range(NCK):
                    t = wp.tile([CK, HP], F32R, name=f"{nm}_{c}", tag=f"{nm}_{c}")
                    nc.sync.dma_start(t[:], src[c * CK:(c + 1) * CK, :])
                    wts[(nm, c)] = t
            for h in range(2):
                nc.sync.dma_start(V[h][:, :, 160:162], onesv[:])

            for sc in range(NSC):
                xts = []
                for c in range(NCK):
                    xt = xp.tile([CK, SC], F32R, tag=f"xt{c}")
                    nc.sync.dma_start(xt[:], xT[c * CK:(c + 1) * CK,
                                                 sc * SC:(sc + 1) * SC])
                    xts.append(xt)
                # QT / KT chunks: psum[m, q] = sum_c w[c, m].T @ xT[c, q]
                for nm, dst in (("wq", None), ("wk", KT)):
                    for i, (off, msz) in enumerate(hchunks[0] + hchunks[1]):
                        ps = pp.tile([msz, SC], F32, tag="ps")
                        for c in range(NCK):
                            nc.tensor.matmul(
                                ps[:], wts[(nm, c)][:, off:off + msz], xts[c][:],
                                start=(c == 0), stop=(c == NCK - 1))
                        st = sp.tile([msz, SC], F32R, tag=f"st{msz}")
                        nc.vector.tensor_copy(st[:], ps[:])
                        if dst is None:
                            nc.sync.dma_start(
                                qT_d[off:off + msz, sc * SC:(sc + 1) * SC], st[:])
                        else:
                            nc.vector.tensor_copy(
                                dst[i][:, sc * SC:(sc + 1) * SC], ps[:])
                # V natural: psum[s, dv] = xT[c, s].T @ wv[c, :]
                for st4 in range(4):
                    s0 = sc * 4 + st4
                    ps = pp.tile([128, HP], F32, tag="psv")
                    for c in range(NCK):
                        nc.tensor.matmul(
                            ps[:], xts[c][:, st4 * 128:(st4 + 1) * 128],
                            wts[("wv", c)][:], start=(c == 0), stop=(c == NCK - 1))
                    for h in range(2):
                        nc.vector.tensor_copy(V[h][:, s0, 0:160],
                                              ps[:, h * 160:(h + 1) * 160])

        # ---- phase 2: attention per head ----
        with ExitStack() as ph2:
            qp = ph2.enter_context(tc.tile_pool(name="qp", bufs=2))
            scp = ph2.enter_context(tc.tile_pool(name="scp", bufs=3, space="PSUM"))
            ovp = ph2.enter_context(tc.tile_pool(name="ovp", bufs=2, space="PSUM"))
            rbp = ph2.enter_context(tc.tile_pool(name="rbp", bufs=1, space="PSUM"))
            ep = ph2.enter_context(tc.tile_pool(name="ep", bufs=3))
            np_ = ph2.enter_context(tc.tile_pool(name="np", bufs=2))
            o2 = ph2.enter_context(tc.tile_pool(name="o2", bufs=1))
            ones2_t = o2.tile([1, 128], F32)
            nc.sync.dma_start(ones2_t[:], ones2[:])

            qts = {}
            for h in range(2):
                (offA, _), (offB, _) = hchunks[h]
                for qc in range(NSC):
                    qs = slice(qc * SC, (qc + 1) * SC)
                    qA = qp.tile([128, SC], F32R, tag=f"qA{h}_{qc}", bufs=1)
                    qB = qp.tile([32, SC], F32R, tag=f"qB{h}_{qc}", bufs=1)
                    nc.sync.dma_start(qA[:], qT_d[offA:offA + 128, qs])
                    nc.sync.dma_start(qB[:], qT_d[offB:offB + 32, qs])
                    qts[(h, qc)] = (qA, qB)

            for h in range(2):
                (offA, _), (offB, _) = hchunks[h]
                ktA, ktB = KT[2 * h], KT[2 * h + 1]
                for qc in range(NSC):
                    qs = slice(qc * SC, (qc + 1) * SC)
                    qA, qB = qts[(h, qc)]
                    oA = ovp.tile([128, SC], F32, tag="oA")
                    oB = ovp.tile([34, SC], F32, tag="oB")
                    for j in range(32):
                        js = slice(j * 128, (j + 1) * 128)
                        sc_ps = scp.tile([128, SC], F32, tag="sc")
                        nc.tensor.matmul(sc_ps[:], ktA[:, js], qA[:],
                                         start=True, stop=False)
                        nc.tensor.matmul(sc_ps[:], ktB[:, js], qB[:],
                                         start=False, stop=True)
                        ex = ep.tile([128, SC], F32R, tag="ex")
                        nc.scalar.activation(ex[:], sc_ps[:],
                                             mybir.ActivationFunctionType.Exp)
                        nc.tensor.matmul(oA[:], V[h][:, j, 0:128], ex[:],
                                         start=(j == 0), stop=(j == 31))
                        nc.tensor.matmul(oB[:], V[h][:, j, 128:162], ex[:],
                                         start=(j == 0), stop=(j == 31))
                    # normalize by denominator (row 32 of oB) and store
                    rec = np_.tile([1, SC], F32, tag="rec")
                    nc.vector.reciprocal(rec[:], oB[32:33, :])
                    rb = rbp.tile([128, SC], F32, tag="rb")
                    nc.tensor.matmul(rb[:], ones2_t[:], rec[:],
                                     start=True, stop=True)
                    rbs = np_.tile([128, SC], F32, tag="rbs")
                    nc.vector.tensor_copy(rbs[:], rb[:])
                    onA = np_.tile([128, SC], F32R, tag="onA")
                    onB = np_.tile([32, SC], F32R, tag="onB")
                    nc.vector.tensor_mul(onA[:], oA[:], rbs[:])
                    nc.vector.tensor_mul(onB[:], oB[0:32, :], rbs[0:32, :])
                    nc.sync.dma_start(oT_d[offA:offA + 128, qs], onA[:])
                    nc.sync.dma_start(oT_d[offB:offB + 32, qs], onB[:])

        # ---- phase 3: output projection (partial over this core's cols) ----
        with ExitStack() as ph3:
            op = ph3.enter_context(tc.tile_pool(name="op", bufs=1))
            wop = ph3.enter_context(tc.tile_pool(name="wop", bufs=1))
            fp = ph3.enter_context(tc.tile_pool(name="fp", bufs=4, space="PSUM"))
            fs = ph3.enter_context(tc.tile_pool(name="fs", bufs=3))
            chunks = hchunks[0] + hchunks[1]
            woc = []
            for i, (off, msz) in enumerate(chunks):
                w = wop.tile([msz, D], F32R, name=f"wo{i}", tag=f"wo{i}")
                nc.sync.dma_start(w[:], wo[off:off + msz, :])
                woc.append(w)
            for qc in range(NSC):
                qs = slice(qc * SC, (qc + 1) * SC)
                oTc = []
                for i, (off, msz) in enumerate(chunks):
                    t = op.tile([msz, SC], F32R, tag=f"oT{i}", bufs=2)
                    nc.sync.dma_start(t[:], oT_d[off:off + msz, qs])
                    oTc.append(t)
                for st4 in range(4):
                    ss = slice(st4 * 128, (st4 + 1) * 128)
                    row = qc * SC + st4 * 128
                    ot = fs.tile([128, D], F32, tag="ot")
                    for oc, osz in ((0, 512), (512, 512), (1024, 256)):
                        ps = fp.tile([128, osz], F32, tag=f"fo{osz}")
                        for i in range(4):
                            nc.tensor.matmul(ps[:], oTc[i][:, ss],
                                             woc[i][:, oc:oc + osz],
                                             start=(i == 0), stop=(i == 3))
                        nc.vector.tensor_copy(ot[:, oc:oc + osz], ps[:])
                    nc.sync.dma_start(out[row:row + 128, :], ot[:])

    nc.compile()
    return nc


def kernel(hidden_states, w_q, w_k, w_v, lora_k_a, lora_k_b,
           lora_v_a, lora_v_b, w_out, b_out):
    f64 = np.float64
    wk_eff = (w_k.astype(f64)
              + w_k.astype(f64) @ lora_k_a.astype(f64) @ lora_k_b.astype(f64)
              ).astype(np.float32)
    wv_eff = (w_v.astype(f64)
              + w_v.astype(f64) @ lora_v_a.astype(f64) @ lora_v_b.astype(f64)
              ).astype(np.float32)
    wq_s = (w_q.astype(f64) / np.sqrt(DH)).astype(np.float32)

    ones2 = np.ones((1, 128), np.float32)
    onesv = np.zeros((128, 32, 2), np.float32)
    onesv[:, :, 0] = 1.0
    xT = [np.ascontiguousarray(np.asarray(hidden_states)[b].T) for b in range(B)]

    in_maps = []
    for c in range(N_CORES):
        b, p = c // 4, c % 4
        cols = slice(p * HP, (p + 1) * HP)
        in_maps.append({
            "xT": xT[b],
            "wq": np.ascontiguousarray(wq_s[:, cols]),
            "wk": np.ascontiguousarray(wk_eff[:, cols]),
            "wv": np.ascontiguousarray(wv_eff[:, cols]),
            "wo": np.ascontiguousarray(w_out[cols, :]),
            "ones2": ones2,
            "onesv": onesv,
        })

    global _last_in_maps
    _last_in_maps = in_maps
    if "nc" not in _CACHE:
        _CACHE["nc"] = build()
    res = run_bass_kernel_spmd(_CACHE["nc"], in_maps, list(range(N_CORES)))

    out = np.zeros((B, S, D), np.float32)
    for c in range(N_CORES):
        out[c // 4] += res.results[c]["out"]
    out += np.asarray(b_out, np.float32)
    return out



# revision 11
# speedup vs baseline: 1.7757x; 1.7757x over previous
"""LoRA attention processor on 8 NeuronCores (Trainium2, Bass/Tile), bf16.

Reference computation (B=2, S=4096, D=1280, H=8 heads, dh=160, rank-4 LoRA
on K/V):
    q = x @ Wq; k = x @ Wk; v = x @ Wv
    k += (k @ Ak) @ Bk; v += (v @ Av) @ Bv        (LoRA, rank 4)
    attn = softmax(q k^T / sqrt(dh)) v   per head
    out = attn @ Wout + b_out

Sharding: core c handles batch b = c//4 and head pair p = c%4 (columns
320p:320p+320 of the QKV projections, rows of Wout). LoRA is folded into
the weights on the host. Each core returns a partial output (its heads'
contribution to attn@Wout); the host sums the 4 partials per batch and
adds the bias.

v2 vs the fp32r baseline: all matmuls run in bf16 (fp32r trips the PE
activity throttler: HAM clamps to K=4/8 = 1.2 GHz for ~86% of the run;
bf16 sustains 2.4 GHz). Everything stays SBUF-resident (no qT/oT DRAM
round-trips). Projections use (128,128,64) M-chunks over the core's 320
columns via a host-side column permutation [h0 d0:128 | h1 d0:128 |
h0 d128:160 | h1 d128:160], so per-head attention slices stay
partition-aligned (leftover d-rows of head h live at partitions 32h:32h+32
of the third chunk). Scores are computed transposed ([k, q]) in 1024-wide
q chunks so each exp activation covers [128,1024] (2 PSUM banks),
amortizing ACT overhead. PV runs in natural layout: out[q, d] accumulates
over k-blocks with lhsT = exp-tile slices, rhs = V[k-block] with a
ones-column appended (row-sum denominator rides along as output column
160). Normalization by 1/denominator is folded into the PSUM evacuation
(ACT copy with per-partition scale), the normalized tiles are transposed
on the PE (cheap) into [d, s] layout, and the output projection contracts
d in (128,128,64) chunks.
"""

import numpy as np
import ml_dtypes
from contextlib import ExitStack

import concourse.bass as bass
import concourse.tile as tile
from concourse import bacc, mybir
from concourse.bass_utils import run_bass_kernel_spmd

B, S, D = 2, 4096, 1280
H, DH = 8, 160
HP = 320           # head-pair columns per core (2 heads)
N_CORES = 8
SC = 512           # phase-1 free-dim chunk
NSC = S // SC      # 8
QC = 1024          # phase-2 q chunk (2 PSUM banks wide)
NQC = S // QC      # 4
CK = 128           # contraction chunk
NCK = D // CK      # 10
NJ = S // 128      # 32 k-blocks
F32 = mybir.dt.float32
BF16 = mybir.dt.bfloat16
EXP = mybir.ActivationFunctionType.Exp

# M-chunks of the permuted 320-wide projection: (offset, size)
CHUNKS = [(0, 128), (128, 128), (256, 64)]

_CACHE = {}


def build():
    nc = bacc.Bacc("TRN2", target_bir_lowering=False, debug=False,
                   num_devices=N_CORES)
    xT = nc.dram_tensor("xT", [D, S], BF16, kind="ExternalInput").ap()
    wq = nc.dram_tensor("wq", [D, HP], BF16, kind="ExternalInput").ap()
    wk = nc.dram_tensor("wk", [D, HP], BF16, kind="ExternalInput").ap()
    wv = nc.dram_tensor("wv", [D, HP], BF16, kind="ExternalInput").ap()
    wo = nc.dram_tensor("wo", [HP, D], BF16, kind="ExternalInput").ap()
    ident = nc.dram_tensor("ident", [128, 128], F32, kind="ExternalInput").ap()
    out = nc.dram_tensor("out", [S, D], F32, kind="ExternalOutput").ap()

    with tile.TileContext(nc) as tc, ExitStack() as top:
        # persistent SBUF tensors
        per = top.enter_context(tc.tile_pool(name="per", bufs=1))
        qt = [per.tile([sz, S], BF16, name=f"qt{i}", tag=f"qt{i}")
              for i, (_, sz) in enumerate(CHUNKS)]
        kt = [per.tile([sz, S], BF16, name=f"kt{i}", tag=f"kt{i}")
              for i, (_, sz) in enumerate(CHUNKS)]
        V = [per.tile([128, NJ, 162], BF16, name=f"V{h}", tag=f"V{h}")
             for h in range(2)]
        oT = [per.tile([sz, S], BF16, name=f"oT{i}", tag=f"oT{i}")
              for i, (_, sz) in enumerate(CHUNKS)]
        id_t = per.tile([128, 128], F32, name="id_t", tag="id_t")
        nc.sync.dma_start(id_t[:], ident[:])
        for h in range(2):
            nc.vector.memset(V[h][:, :, 160:162], 0.0)
            nc.vector.memset(V[h][:, :, 160:161], 1.0)

        # ---- phase 1: projections Q/K (transposed chunks) + V (natural) ----
        with ExitStack() as ph1:
            xp = ph1.enter_context(tc.tile_pool(name="xp", bufs=2))
            wp = ph1.enter_context(tc.tile_pool(name="wp", bufs=1))
            pp = ph1.enter_context(tc.tile_pool(name="pp", bufs=4, space="PSUM"))
            sp = ph1.enter_context(tc.tile_pool(name="sp", bufs=2))

            warm = sp.tile([1, 2], F32, tag="warm")
            nc.vector.memset(warm[:], 0.0)
            warm2 = sp.tile([1, 2], F32, tag="warm2")
            nc.scalar.activation(warm2[:], warm[:], EXP)

            wts = {}
            for nm, src in (("wq", wq), ("wk", wk), ("wv", wv)):
                for c in range(NCK):
                    t = wp.tile([CK, HP], BF16, name=f"{nm}_{c}", tag=f"{nm}_{c}")
                    nc.sync.dma_start(t[:], src[c * CK:(c + 1) * CK, :])
                    wts[(nm, c)] = t

            for sc in range(NSC):
                ss = slice(sc * SC, (sc + 1) * SC)
                xts = []
                for c in range(NCK):
                    xt = xp.tile([CK, SC], BF16, tag=f"xt{c}")
                    nc.sync.dma_start(xt[:], xT[c * CK:(c + 1) * CK, ss])
                    xts.append(xt)
                # Q/K transposed: psum[m, s] = w[c, m].T @ xT[c, s]
                for nm, dst in (("wq", qt), ("wk", kt)):
                    for i, (off, msz) in enumerate(CHUNKS):
                        ps = pp.tile([msz, SC], F32, tag="ps")
                        for c in range(NCK):
                            nc.tensor.matmul(
                                ps[:], wts[(nm, c)][:, off:off + msz], xts[c][:],
                                start=(c == 0), stop=(c == NCK - 1))
                        nc.vector.tensor_copy(dst[i][:, ss], ps[:])
                # V natural: psum[s, dv] = xT[c, s].T @ wv[c, :]
                for st4 in range(4):
                    s0 = sc * 4 + st4
                    ps = pp.tile([128, HP], F32, tag="psv")
                    for c in range(NCK):
                        nc.tensor.matmul(
                            ps[:], xts[c][:, st4 * 128:(st4 + 1) * 128],
                            wts[("wv", c)][:], start=(c == 0), stop=(c == NCK - 1))
                    for h in range(2):
                        nc.vector.tensor_copy(V[h][:, s0, 0:160],
                                              ps[:, h * 160:(h + 1) * 160])

        # ---- phase 2+3: attention + output projection, per 1024-q chunk ----
        # PSUM budget (8 banks): "sc"-tagged [128,1024]f32 tiles (2 banks x
        # 2 bufs = 4) host the score matmuls AND (via disjoint slices) the
        # transpose outputs and phase-3 accumulators; pv tiles take 3 banks.
        with ExitStack() as ph2:
            big = ph2.enter_context(tc.tile_pool(name="big", bufs=2, space="PSUM"))
            pvp = ph2.enter_context(tc.tile_pool(name="pvp", bufs=1, space="PSUM"))
            exp_ = ph2.enter_context(tc.tile_pool(name="exq", bufs=2))
            nap = ph2.enter_context(tc.tile_pool(name="nap", bufs=1))
            nnp = ph2.enter_context(tc.tile_pool(name="nnp", bufs=1))
            rcp = ph2.enter_context(tc.tile_pool(name="rcp", bufs=1))
            wop = ph2.enter_context(tc.tile_pool(name="wop", bufs=1))
            fsp = ph2.enter_context(tc.tile_pool(name="fsp", bufs=2))

            woc = []
            for i, (off, msz) in enumerate(CHUNKS):
                w = wop.tile([msz, D], BF16, name=f"wo{i}", tag=f"wo{i}")
                nc.sync.dma_start(w[:], wo[off:off + msz, :])
                woc.append(w)

            # per-head score operands: (K=128 tile, leftover slice base)
            def score_ops(h):
                ktA = kt[h]          # [128, S]
                qtA = qt[h]
                ktB = kt[2][32 * h:32 * h + 32, :]   # [32, S] at base 32h
                qtB = qt[2][32 * h:32 * h + 32, :]
                return ktA, qtA, ktB, qtB

            # pv psum slot for a q-subtile (0..7): 3+3+2 per bank
            def pv_slot(pvt, ql):
                if ql < 3:
                    return pvt[0][:, ql]
                if ql < 6:
                    return pvt[1][:, ql - 3]
                return pvt[2][:, ql - 6]

            for qc in range(NQC):
                q0 = qc * QC
                tasks = [(h, j) for h in range(2) for j in range(NJ)]
                pvt = [pvp.tile([128, 3, 162], F32, tag="pv0", name="pv0"),
                       pvp.tile([128, 3, 162], F32, tag="pv1", name="pv1"),
                       pvp.tile([128, 2, 162], F32, tag="pv2", name="pv2")]
                scs, exs = {}, {}

                def emit_scores(h, j):
                    ktA, qtA, ktB, qtB = score_ops(h)
                    js = slice(j * 128, (j + 1) * 128)
                    sc_ps = big.tile([128, QC], F32, tag="sc")
                    for half in range(2):
                        qs = slice(q0 + half * 512, q0 + (half + 1) * 512)
                        dst = sc_ps[:, half * 512:(half + 1) * 512]
                        nc.tensor.matmul(dst, ktA[:, js], qtA[:, qs],
                                         start=True, stop=False)
                        nc.tensor.matmul(dst, ktB[:, js], qtB[:, qs],
                                         start=False, stop=True)
                    ex = exp_.tile([128, QC], BF16, tag="ex")
                    nc.scalar.activation(ex[:], sc_ps[:], EXP)
                    scs[(h, j)], exs[(h, j)] = sc_ps, ex

                def emit_pv(h, j):
                    ex = exs.pop((h, j))
                    scs.pop((h, j))
                    for ql in range(8):
                        # start=True clears has_written bits BANK-wide, so
                        # only the first matmul in each bank may use it; the
                        # other groups in the bank start with cleared bits
                        # (overwrite) and accumulate from j=1 on.
                        st = (j == 0) and ql in (0, 3, 6)
                        nc.tensor.matmul(
                            pv_slot(pvt, ql), ex[:, ql * 128:(ql + 1) * 128],
                            V[h][:, j, :], start=st, stop=(j == NJ - 1),
                            skip_group_check=True)

                natA = {}
                natN = [nnp.tile([128, 64], F32, tag=f"nn{ql}",
                                 name=f"nn{ql}")
                        for ql in range(8)]

                def emit_evac(h):
                    for ql in range(8):
                        pv = pv_slot(pvt, ql)
                        rec = rcp.tile([128, 1], F32, tag=f"rc{h}_{ql}")
                        nc.vector.reciprocal(rec[:], pv[:, 160:161])
                        na = nap.tile([128, 128], F32, tag=f"na{h}_{ql}")
                        nc.scalar.mul(na[:], pv[:, 0:128], rec[:])
                        nc.scalar.mul(natN[ql][:, 32 * h:32 * h + 32],
                                      pv[:, 128:160], rec[:])
                        natA[(h, ql)] = na

                emit_scores(*tasks[0])
                for i, (h, j) in enumerate(tasks):
                    if i + 1 < len(tasks):
                        emit_scores(*tasks[i + 1])
                    emit_pv(h, j)
                    if j == NJ - 1:
                        emit_evac(h)

                # transposes: natural [q, d] -> oT chunks [d, q] (f32 in PE
                # transpose mode; outputs land in slices of an "sc" buffer)
                for ql in range(8):
                    qg = slice(q0 + ql * 128, q0 + (ql + 1) * 128)
                    pt = big.tile([128, QC], F32, tag="sc")
                    nc.tensor.transpose(pt[:, 0:128], natA[(0, ql)][:], id_t[:])
                    nc.tensor.transpose(pt[:, 128:256], natA[(1, ql)][:],
                                        id_t[:])
                    nc.tensor.transpose(pt[0:64, 256:384], natN[ql][:], id_t[:])
                    nc.vector.tensor_copy(oT[0][:, qg], pt[:, 0:128])
                    nc.vector.tensor_copy(oT[1][:, qg], pt[:, 128:256])
                    nc.vector.tensor_copy(oT[2][:, qg], pt[0:64, 256:384])

                # output projection for this q chunk
                for sb in range(8):
                    row = q0 + sb * 128
                    sbs = slice(row, row + 128)
                    fs = fsp.tile([128, D], F32, tag="fs")
                    for oi, (oc, osz) in enumerate(((0, 512), (512, 512),
                                                    (1024, 256))):
                        ps = big.tile([128, QC], F32, tag="sc")
                        for i in range(3):
                            nc.tensor.matmul(ps[:, 0:osz], oT[i][:, sbs],
                                             woc[i][:, oc:oc + osz],
                                             start=(i == 0), stop=(i == 2))
                        if oi % 2 == 0:
                            nc.vector.tensor_copy(fs[:, oc:oc + osz],
                                                  ps[:, 0:osz])
                        else:
                            nc.scalar.copy(fs[:, oc:oc + osz], ps[:, 0:osz])
                    nc.sync.dma_start(out[sbs, :], fs[:])

    nc.compile()
    return nc


def kernel(hidden_states, w_q, w_k, w_v, lora_k_a, lora_k_b,
           lora_v_a, lora_v_b, w_out, b_out):
    f64 = np.float64
    bf16 = ml_dtypes.bfloat16
    wk_eff = (w_k.astype(f64)
              + w_k.astype(f64) @ lora_k_a.astype(f64) @ lora_k_b.astype(f64)
              ).astype(np.float32)
    wv_eff = (w_v.astype(f64)
              + w_v.astype(f64) @ lora_v_a.astype(f64) @ lora_v_b.astype(f64)
              ).astype(np.float32)
    wq_s = (w_q.astype(f64) / np.sqrt(DH)).astype(np.float32)

    ident = np.eye(128, dtype=np.float32)
    xT = [np.ascontiguousarray(np.asarray(hidden_states)[b].T).astype(bf16)
          for b in range(B)]

    in_maps = []
    for c in range(N_CORES):
        b, p = c // 4, c % 4
        ha, hb = 2 * p, 2 * p + 1
        # permuted columns: [h0 d0:128 | h1 d0:128 | h0 d128:160 | h1 d128:160]
        perm = np.concatenate([
            np.arange(ha * DH, ha * DH + 128),
            np.arange(hb * DH, hb * DH + 128),
            np.arange(ha * DH + 128, (ha + 1) * DH),
            np.arange(hb * DH + 128, (hb + 1) * DH)])
        cols = slice(p * HP, (p + 1) * HP)
        in_maps.append({
            "xT": xT[b],
            "wq": np.ascontiguousarray(wq_s[:, perm]).astype(bf16),
            "wk": np.ascontiguousarray(wk_eff[:, perm]).astype(bf16),
            "wv": np.ascontiguousarray(wv_eff[:, cols]).astype(bf16),
            "wo": np.ascontiguousarray(w_out[perm, :]).astype(bf16),
            "ident": ident,
        })

    global _last_in_maps
    _last_in_maps = in_maps
    if "nc" not in _CACHE:
        _CACHE["nc"] = build()
    res = run_bass_kernel_spmd(_CACHE["nc"], in_maps, list(range(N_CORES)))

    out = np.zeros((B, S, D), np.float32)
    for c in range(N_CORES):
        out[c // 4] += res.results[c]["out"]
    out += np.asarray(b_out, np.float32)
    return out


# revision 12
# speedup vs baseline: 2.6790x; 1.5087x over previous
"""LoRA attention processor on 8 NeuronCores (Trainium2, Bass/Tile), bf16.

Reference computation (B=2, S=4096, D=1280, H=8 heads, dh=160, rank-4 LoRA
on K/V):
    q = x @ Wq; k = x @ Wk; v = x @ Wv
    k += (k @ Ak) @ Bk; v += (v @ Av) @ Bv        (LoRA, rank 4)
    attn = softmax(q k^T / sqrt(dh)) v   per head
    out = attn @ Wout + b_out

Sharding: core c handles batch b = c//4 and head pair p = c%4 (columns
320p:320p+320 of the QKV projections, rows of Wout). LoRA is folded into
the weights on the host. Each core returns a partial output (its heads'
contribution to attn@Wout); the host sums the 4 partials per batch and
adds the bias.

Design notes (all empirically driven; the PE gets power-clamped to
1.2 GHz after ~220us of sustained 8-core matmul activity, so the kernel
is column-count-bound):
- All matmuls in bf16 (fp32r keeps the clamp at ~50% util for the whole
  run); everything SBUF-resident.
- Projections: per-head d0:128 chunks of Q and K are produced as [128,S]
  tiles; the two heads' leftover d128:160 dims of BOTH q and k are fused
  into one 128-wide M-chunk (host column permutation), then a SBUF->SBUF
  DMA builds a half-swapped duplicate so that k3/q3 coexist in strips
  {32h} and {64+32h} - the two K=32 leftover score matmuls of a q-chunk
  then run CONCURRENTLY on disjoint 32-row PE tiles (different PSUM
  banks; same-bank concurrency is a hardware fault).
- Scores are computed transposed ([k,q]) in 1024-wide q chunks, j-pairs
  batched so 128-row-mode and 32-row-mode matmuls alternate once per j
  instead of twice (mode switches drain the PE). exp runs on ACT over
  [128,1024] (2 PSUM banks per instruction, halving ACT overhead).
- PV runs in natural layout out[q,d]: lhsT = exp-tile slices, rhs =
  V[k-block] with a ones-column appended (denominator rides along as
  column 160). PSUM start=True clears has_written bits BANK-wide, so only
  the first matmul of each bank uses start=True; the other interleaved
  accumulation groups in that bank begin with start=False (cleared bits
  -> overwrite) - verified on hardware.
- Normalization is folded into PSUM evacuation (ACT copy with
  per-partition 1/denom scale), the normalized tiles are transposed on
  the PE, and the output projection contracts d in (128,128,64) chunks.
"""

import numpy as np
import ml_dtypes
from contextlib import ExitStack

import concourse.bass as bass
import concourse.tile as tile
from concourse import bacc, mybir
from concourse.bass_utils import run_bass_kernel_spmd

B, S, D = 2, 4096, 1280
H, DH = 8, 160
HP = 320           # head-pair columns per core (2 heads)
N_CORES = 8
SC = 512           # phase-1 free-dim chunk
NSC = S // SC      # 8
QC = 1024          # phase-2 q chunk (2 PSUM banks wide)
NQC = S // QC      # 4
CK = 128           # contraction chunk
NCK = D // CK      # 10
NJ = S // 128      # 32 k-blocks
F32 = mybir.dt.float32
BF16 = mybir.dt.bfloat16
EXP = mybir.ActivationFunctionType.Exp

CHUNKS = [(0, 128), (128, 128), (256, 64)]   # oT / wo row chunks

_CACHE = {}


def build():
    nc = bacc.Bacc("TRN2", target_bir_lowering=False, debug=False,
                   num_devices=N_CORES)
    xT = nc.dram_tensor("xT", [D, S], BF16, kind="ExternalInput").ap()
    wq = nc.dram_tensor("wq", [D, 256], BF16, kind="ExternalInput").ap()
    wk = nc.dram_tensor("wk", [D, 256], BF16, kind="ExternalInput").ap()
    wqk = nc.dram_tensor("wqk", [D, 128], BF16, kind="ExternalInput").ap()
    wv = nc.dram_tensor("wv", [D, HP], BF16, kind="ExternalInput").ap()
    wo = nc.dram_tensor("wo", [HP, D], BF16, kind="ExternalInput").ap()
    ident = nc.dram_tensor("ident", [128, 128], F32, kind="ExternalInput").ap()
    out = nc.dram_tensor("out", [S, D], F32, kind="ExternalOutput").ap()

    with tile.TileContext(nc) as tc, ExitStack() as top:
        # persistent SBUF tensors
        per = top.enter_context(tc.tile_pool(name="per", bufs=1))
        qt = [per.tile([128, S], BF16, name=f"qt{i}", tag=f"qt{i}")
              for i in range(2)]
        kt = [per.tile([128, S], BF16, name=f"kt{i}", tag=f"kt{i}")
              for i in range(2)]
        # qkb rows 0:64 = q3 [h0|h1], rows 64:128 = k3 [h0|h1]; qkd swapped
        qkb = per.tile([128, S], BF16, name="qkb", tag="qkb")
        qkd = per.tile([128, S], BF16, name="qkd", tag="qkd")
        V = [per.tile([128, NJ, 162], BF16, name=f"V{h}", tag=f"V{h}")
             for h in range(2)]
        oT = [per.tile([sz, S], BF16, name=f"oT{i}", tag=f"oT{i}")
              for i, (_, sz) in enumerate(CHUNKS)]
        id_t = per.tile([128, 128], F32, name="id_t", tag="id_t")
        nc.sync.dma_start(id_t[:], ident[:])
        for h in range(2):
            nc.vector.memset(V[h][:, :, 160:162], 0.0)
            nc.vector.memset(V[h][:, :, 160:161], 1.0)

        # ---- phase 1: projections Q/K (transposed chunks) + V (natural) ----
        with ExitStack() as ph1:
            xp = ph1.enter_context(tc.tile_pool(name="xp", bufs=2))
            wp = ph1.enter_context(tc.tile_pool(name="wp", bufs=1))
            pp = ph1.enter_context(tc.tile_pool(name="pp", bufs=4, space="PSUM"))
            sp = ph1.enter_context(tc.tile_pool(name="sp", bufs=2))

            warm = sp.tile([1, 2], F32, tag="warm")
            nc.vector.memset(warm[:], 0.0)
            warm2 = sp.tile([1, 2], F32, tag="warm2")
            nc.scalar.activation(warm2[:], warm[:], EXP)

            wts = {}
            for nm, src, w_ in (("wq", wq, 256), ("wk", wk, 256),
                                ("wqk", wqk, 128), ("wv", wv, HP)):
                for c in range(NCK):
                    t = wp.tile([CK, w_], BF16, name=f"{nm}_{c}",
                                tag=f"{nm}_{c}")
                    nc.sync.dma_start(t[:], src[c * CK:(c + 1) * CK, :])
                    wts[(nm, c)] = t

            for sc in range(NSC):
                ss = slice(sc * SC, (sc + 1) * SC)
                xts = []
                for c in range(NCK):
                    xt = xp.tile([CK, SC], BF16, tag=f"xt{c}")
                    nc.sync.dma_start(xt[:], xT[c * CK:(c + 1) * CK, ss])
                    xts.append(xt)
                # transposed projections: psum[m, s] = w[c, m].T @ xT[c, s]
                groups = [("wq", 0, qt[0]), ("wq", 128, qt[1]),
                          ("wk", 0, kt[0]), ("wk", 128, kt[1]),
                          ("wqk", 0, qkb)]
                for nm, off, dst in groups:
                    ps = pp.tile([128, SC], F32, tag="ps")
                    for c in range(NCK):
                        nc.tensor.matmul(
                            ps[:], wts[(nm, c)][:, off:off + 128], xts[c][:],
                            start=(c == 0), stop=(c == NCK - 1))
                    nc.vector.tensor_copy(dst[:, ss], ps[:])
                # V natural: psum[s, dv] = xT[c, s].T @ wv[c, :]
                for st4 in range(4):
                    s0 = sc * 4 + st4
                    ps = pp.tile([128, HP], F32, tag="psv")
                    for c in range(NCK):
                        nc.tensor.matmul(
                            ps[:], xts[c][:, st4 * 128:(st4 + 1) * 128],
                            wts[("wv", c)][:], start=(c == 0),
                            stop=(c == NCK - 1))
                    for h in range(2):
                        nc.vector.tensor_copy(V[h][:, s0, 0:160],
                                              ps[:, h * 160:(h + 1) * 160])

            # half-swapped duplicate of qkb (SBUF->SBUF DMA, shifts
            # partitions): qkd[0:64]=k3, qkd[64:128]=q3
            nc.sync.dma_start(qkd[0:64, :], qkb[64:128, :])
            nc.sync.dma_start(qkd[64:128, :], qkb[0:64, :])

        # ---- phase 2+3: attention + output projection, per 1024-q chunk ----
        # PSUM budget (8 banks): "sc"-tagged [128,1024]f32 tiles (2 banks x
        # 2 bufs = 4) host the score matmuls AND (via disjoint slices) the
        # transpose outputs and phase-3 accumulators; pv tiles take 3 banks.
        with ExitStack() as ph2:
            big = ph2.enter_context(tc.tile_pool(name="big", bufs=2,
                                                 space="PSUM"))
            pvp = ph2.enter_context(tc.tile_pool(name="pvp", bufs=1,
                                                 space="PSUM"))
            exp_ = ph2.enter_context(tc.tile_pool(name="exq", bufs=4))
            nap = ph2.enter_context(tc.tile_pool(name="nap", bufs=1))
            nnp = ph2.enter_context(tc.tile_pool(name="nnp", bufs=1))
            rcp = ph2.enter_context(tc.tile_pool(name="rcp", bufs=1))
            wop = ph2.enter_context(tc.tile_pool(name="wop", bufs=1))
            fsp = ph2.enter_context(tc.tile_pool(name="fsp", bufs=2))

            woc = []
            for i, (off, msz) in enumerate(CHUNKS):
                w = wop.tile([msz, D], BF16, name=f"wo{i}", tag=f"wo{i}")
                nc.sync.dma_start(w[:], wo[off:off + msz, :])
                woc.append(w)

            # pv psum slot for a q-subtile (0..7): 3+3+2 per bank
            def pv_slot(pvt, ql):
                if ql < 3:
                    return pvt[0][:, ql]
                if ql < 6:
                    return pvt[1][:, ql - 3]
                return pvt[2][:, ql - 6]

            for qc in range(NQC):
                q0 = qc * QC
                pairs = [(h, jp) for h in range(2) for jp in range(0, NJ, 2)]
                pvt = [pvp.tile([128, 3, 162], F32, tag="pv0", name="pv0"),
                       pvp.tile([128, 3, 162], F32, tag="pv1", name="pv1"),
                       pvp.tile([128, 2, 162], F32, tag="pv2", name="pv2")]
                exs = {}

                def emit_scores(h, jp):
                    """Scores for j-pair (jp, jp+1): 128-mode matmuls batched
                    before 32-mode ones (1 mode switch per j instead of 2);
                    the two K=32 leftovers of each j run concurrently on
                    disjoint 32-row tiles / different PSUM banks."""
                    scp = [big.tile([128, QC], F32, tag="sc", name="sca"),
                           big.tile([128, QC], F32, tag="sc", name="scb")]
                    for jj in range(2):
                        js = slice((jp + jj) * 128, (jp + jj + 1) * 128)
                        for half in range(2):
                            qs = slice(q0 + half * 512, q0 + (half + 1) * 512)
                            nc.tensor.matmul(
                                scp[jj][:, half * 512:(half + 1) * 512],
                                kt[h][:, js], qt[h][:, qs],
                                start=True, stop=False)
                    for jj in range(2):
                        js = slice((jp + jj) * 128, (jp + jj + 1) * 128)
                        qs0 = slice(q0, q0 + 512)
                        qs1 = slice(q0 + 512, q0 + 1024)
                        nc.tensor.matmul(
                            scp[jj][:, 0:512],
                            qkd[32 * h:32 * h + 32, js],
                            qkb[32 * h:32 * h + 32, qs0],
                            start=False, stop=True,
                            tile_position=(32 * h, 0))
                        nc.tensor.matmul(
                            scp[jj][:, 512:1024],
                            qkb[64 + 32 * h:96 + 32 * h, js],
                            qkd[64 + 32 * h:96 + 32 * h, qs1],
                            start=False, stop=True,
                            tile_position=(64 + 32 * h, 0))
                    for jj in range(2):
                        ex = exp_.tile([128, QC], BF16, tag="ex")
                        nc.scalar.activation(ex[:], scp[jj][:], EXP)
                        exs[(h, jp + jj)] = ex

                def emit_pv(h, jp):
                    for jj in range(2):
                        j = jp + jj
                        ex = exs.pop((h, j))
                        for ql in range(8):
                            # start=True clears has_written BANK-wide: only
                            # the first matmul per bank may use it.
                            st = (j == 0) and ql in (0, 3, 6)
                            nc.tensor.matmul(
                                pv_slot(pvt, ql),
                                ex[:, ql * 128:(ql + 1) * 128],
                                V[h][:, j, :], start=st, stop=(j == NJ - 1),
                                skip_group_check=True)

                natA = {}
                natN = [nnp.tile([128, 64], F32, tag=f"nn{ql}",
                                 name=f"nn{ql}")
                        for ql in range(8)]

                def emit_evac(h):
                    for ql in range(8):
                        pv = pv_slot(pvt, ql)
                        rec = rcp.tile([128, 1], F32, tag=f"rc{h}_{ql}",
                                       name="rec")
                        nc.vector.reciprocal(rec[:], pv[:, 160:161])
                        na = nap.tile([128, 128], F32, tag=f"na{h}_{ql}",
                                      name="na")
                        nc.scalar.mul(na[:], pv[:, 0:128], rec[:])
                        nc.scalar.mul(natN[ql][:, 32 * h:32 * h + 32],
                                      pv[:, 128:160], rec[:])
                        natA[(h, ql)] = na

                emit_scores(*pairs[0])
                for i, (h, jp) in enumerate(pairs):
                    if i + 1 < len(pairs):
                        emit_scores(*pairs[i + 1])
                    emit_pv(h, jp)
                    if jp == NJ - 2:
                        emit_evac(h)

                # transposes: natural [q, d] -> oT chunks [d, q] (f32 PE
                # transpose mode; outputs land in slices of an "sc" buffer)
                for ql in range(8):
                    qg = slice(q0 + ql * 128, q0 + (ql + 1) * 128)
                    pt = big.tile([128, QC], F32, tag="sc", name="pt")
                    nc.tensor.transpose(pt[:, 0:128], natA[(0, ql)][:],
                                        id_t[:])
                    nc.tensor.transpose(pt[:, 128:256], natA[(1, ql)][:],
                                        id_t[:])
                    nc.tensor.transpose(pt[0:64, 256:384], natN[ql][:],
                                        id_t[:])
                    nc.vector.tensor_copy(oT[0][:, qg], pt[:, 0:128])
                    nc.vector.tensor_copy(oT[1][:, qg], pt[:, 128:256])
                    nc.vector.tensor_copy(oT[2][:, qg], pt[0:64, 256:384])

                # output projection for this q chunk
                for sb in range(8):
                    row = q0 + sb * 128
                    sbs = slice(row, row + 128)
                    fs = fsp.tile([128, D], F32, tag="fs", name="fs")
                    for oi, (oc, osz) in enumerate(((0, 512), (512, 512),
                                                    (1024, 256))):
                        ps = big.tile([128, QC], F32, tag="sc", name="fo")
                        for i in range(3):
                            nc.tensor.matmul(ps[:, 0:osz], oT[i][:, sbs],
                                             woc[i][:, oc:oc + osz],
                                             start=(i == 0), stop=(i == 2))
                        if oi % 2 == 0:
                            nc.vector.tensor_copy(fs[:, oc:oc + osz],
                                                  ps[:, 0:osz])
                        else:
                            nc.scalar.copy(fs[:, oc:oc + osz], ps[:, 0:osz])
                    nc.sync.dma_start(out[sbs, :], fs[:])

    nc.compile()
    return nc


def kernel(hidden_states, w_q, w_k, w_v, lora_k_a, lora_k_b,
           lora_v_a, lora_v_b, w_out, b_out):
    f64 = np.float64
    bf16 = ml_dtypes.bfloat16
    wk_eff = (w_k.astype(f64)
              + w_k.astype(f64) @ lora_k_a.astype(f64) @ lora_k_b.astype(f64)
              ).astype(np.float32)
    wv_eff = (w_v.astype(f64)
              + w_v.astype(f64) @ lora_v_a.astype(f64) @ lora_v_b.astype(f64)
              ).astype(np.float32)
    wq_s = (w_q.astype(f64) / np.sqrt(DH)).astype(np.float32)

    ident = np.eye(128, dtype=np.float32)
    xT = [np.ascontiguousarray(np.asarray(hidden_states)[b].T).astype(bf16)
          for b in range(B)]

    in_maps = []
    for c in range(N_CORES):
        b, p = c // 4, c % 4
        ha, hb = 2 * p, 2 * p + 1
        mainq = np.concatenate([np.arange(ha * DH, ha * DH + 128),
                                np.arange(hb * DH, hb * DH + 128)])
        left = np.concatenate([np.arange(ha * DH + 128, (ha + 1) * DH),
                               np.arange(hb * DH + 128, (hb + 1) * DH)])
        # wo rows follow the oT layout: [h0 d0:128 | h1 d0:128 | leftovers]
        perm = np.concatenate([mainq, left])
        cols = slice(p * HP, (p + 1) * HP)
        in_maps.append({
            "xT": xT[b],
            "wq": np.ascontiguousarray(wq_s[:, mainq]).astype(bf16),
            "wk": np.ascontiguousarray(wk_eff[:, mainq]).astype(bf16),
            "wqk": np.ascontiguousarray(
                np.concatenate([wq_s[:, left], wk_eff[:, left]],
                               axis=1)).astype(bf16),
            "wv": np.ascontiguousarray(wv_eff[:, cols]).astype(bf16),
            "wo": np.ascontiguousarray(w_out[perm, :]).astype(bf16),
            "ident": ident,
        })

    global _last_in_maps
    _last_in_maps = in_maps
    if "nc" not in _CACHE:
        _CACHE["nc"] = build()
    res = run_bass_kernel_spmd(_CACHE["nc"], in_maps, list(range(N_CORES)))

    out = np.zeros((B, S, D), np.float32)
    for c in range(N_CORES):
        out[c // 4] += res.results[c]["out"]
    out += np.asarray(b_out, np.float32)
    return out
